# revision 8
# baseline (speedup 1.0000x reference)
"""Trainium2 Bass kernel for nn_CumulativeFlattenedLinear.

reference:
  y = fq_out( causal_conv1d(x, fq8(w).reshape(O,C,K), extra_delay=n_discard)
              + fq16(b) )
with power-of-2-scale fake quantization.

Sharding: data-parallel over batch B=16 -> 2 batches per core on 8 cores.
Weight (2 MB) replicated.

On-device compute per core:
  - conv as PE matmuls, contract dim 128 = 64 ch x 2 adjacent taps.
    Partitions 0..63 of the rhs tile hold x's window; partitions 64..127
    hold the same window shifted by one sample, so tap pair (2p, 2p+1)
    is one matmul whose rhs is a column-slice of a single SBUF tile.
    16 tap-pairs accumulate into one PSUM bank per [128 o x 512 t] tile.
  - weights are 8-bit power-of-2 fake-quantized: their integer codes are
    always exact in fp16. x is split x = hi + lo (fp16 each, ~22 mantissa
    bits). int8 x fp16 products are exact in the PE multiplier and
    accumulate in fp32 PSUM, so two fp16 passes reproduce an fp32 conv to
    accumulation-rounding accuracy at half the PE cost of native fp32
    (which takes 4 array passes).
  - epilogue on ACT: out = psum * 2^quanta_w + qbias; DVE tracks
    per-partition running max/min of y for the output requant scale.

Host: weight/bias fake-quant (tiny), final max combine across cores, and
the output requant clip - a no-op unless max|y| sits within 2^-15 of a
power of two (then applied exactly with np.minimum).
"""

import numpy as np

B, C, T, O, K = 16, 64, 8192, 256, 32
N_CORES = 8
BS = B // N_CORES          # batches per core
NT = 512                   # time-tile (one PSUM bank of fp32)
TT = T // NT               # time tiles
OB = O // 128              # output-channel blocks
NPAIR = K // 2             # tap pairs per output tile
NQ = OB * NPAIR            # distinct 128x128 weight blocks
W_WIN = NT + K - 2         # x window columns needed per time tile (542)

_COMPILED: dict = {}


def _ceil_log2_f32(m: np.float32) -> float:
    # exact ceil(log2(m)) for finite positive fp32 m, matching
    # jnp.ceil(jnp.log2(m)) for every case where log2 is correctly rounded
    mant, ex = np.frexp(np.float32(m))  # m = mant * 2^ex, mant in [0.5, 1)
    return float(ex - 1) if mant == 0.5 else float(ex)


def _fake_quant_params(w: np.ndarray, bits: int):
    """Return (q_codes_f32, scale_f32) mirroring reference.fake_quantize:
    fq = clip(round(w/scale), lo, hi) * scale, scale = 2^(ceil(log2(max|w|+1e-12)) - (bits-1))."""
    w32 = np.asarray(w, np.float32)
    maxabs = np.float32(np.max(np.abs(w32)))
    quanta = _ceil_log2_f32(np.float32(maxabs + np.float32(1e-12))) - (bits - 1)
    scale = np.float32(np.exp2(np.float32(quanta)))
    q = (w32 / scale).astype(np.float32)
    q = np.round(q)  # RNE, same as jnp.round
    lo = float(-(2 ** (bits - 1)))
    hi = float(2 ** (bits - 1) - 1)
    q = np.clip(q, lo, hi).astype(np.float32)
    return q, scale


def _build(pad: int, scale_w: float):
    from contextlib import ExitStack

    import concourse.bass as bass
    import concourse.tile as tile
    from concourse import bacc, mybir

    f32 = mybir.dt.float32
    f16 = mybir.dt.float16

    nc = bacc.Bacc(
        "TRN2",
        target_bir_lowering=False,
        debug=False,
        enable_asserts=False,
        num_devices=N_CORES,
    )

    x_d = nc.dram_tensor("x", [BS, C, T], f32, kind="ExternalInput").ap()
    w_d = nc.dram_tensor("wts", [NQ // 4, 128, 512], f16, kind="ExternalInput").ap()
    b_d = nc.dram_tensor("qb", [128, OB], f32, kind="ExternalInput").ap()
    y_d = nc.dram_tensor("y", [BS, O, T], f32, kind="ExternalOutput").ap()
    mm_d = nc.dram_tensor("mm", [128, 2], f32, kind="ExternalOutput").ap()

    NTILES = BS * TT * OB  # output tiles -> columns in the max/min accumulators

    with tile.TileContext(nc) as tc, ExitStack() as ctx:
        wpool = ctx.enter_context(tc.tile_pool(name="w", bufs=1))
        bpool = ctx.enter_context(tc.tile_pool(name="b", bufs=1))
        mpool = ctx.enter_context(tc.tile_pool(name="mx", bufs=1))
        xpool = ctx.enter_context(tc.tile_pool(name="x", bufs=3))
        hpool = ctx.enter_context(tc.tile_pool(name="xh", bufs=3))
        lpool = ctx.enter_context(tc.tile_pool(name="xl", bufs=3))
        opool = ctx.enter_context(tc.tile_pool(name="out", bufs=4))
        pspool = ctx.enter_context(tc.tile_pool(name="ps", bufs=4, space="PSUM"))

        def load_x(b, t):
            t0 = t * NT
            xt = xpool.tile([128, W_WIN], f32, tag="xt")
            # rows 0..63   <- x[b, :, t0-pad   : t0-pad+W_WIN]
            # rows 64..127 <- x[b, :, t0-pad+1 : t0-pad+1+W_WIN]
            lo0 = t0 - pad
            # x loads go via gpsimd (SWDGE queues) so they never queue behind
            # the weight/output transfers on the HWDGE pool.
            if lo0 < 0:
                nc.vector.memset(xt[:], 0.0)
                nc.gpsimd.dma_start(xt[0:64, -lo0:W_WIN], x_d[b, :, 0:W_WIN + lo0])
                nc.gpsimd.dma_start(xt[64:128, -lo0 - 1:W_WIN], x_d[b, :, 0:W_WIN + lo0 + 1])
            else:
                nc.gpsimd.dma_start(xt[0:64, :], x_d[b, :, lo0:lo0 + W_WIN])
                nc.gpsimd.dma_start(xt[64:128, :], x_d[b, :, lo0 + 1:lo0 + 1 + W_WIN])
            xh = hpool.tile([128, W_WIN], f16, tag="xh")
            nc.vector.tensor_copy(xh[:], xt[:])
            xl = lpool.tile([128, W_WIN], f16, tag="xl")
            nc.vector.tensor_tensor(xl[:], xt[:], xh[:], mybir.AluOpType.subtract)
            return xh, xl

        # First x tiles before/between the weight DMAs so the PE can start
        # early; weights land in 4-block chunks (fine-grained deps) ordered
        # so each chunk's transfer beats its first matmul.
        first_xhl = load_x(0, 0)
        wsb = wpool.tile([128, NQ * 128], f16)
        nc.sync.dma_start(wsb[:, 0:512], w_d[0])
        nc.sync.dma_start(wsb[:, 512:1024], w_d[1])
        bsb = bpool.tile([128, OB], f32)
        nc.sync.dma_start(bsb[:], b_d[:])
        second_xhl = load_x(0, 1)
        for cq in range(2, NQ // 4):
            nc.sync.dma_start(wsb[:, cq * 512:(cq + 1) * 512], w_d[cq])

        # per-batch running max/min columns, folded per batch then at the end
        mxh = mpool.tile([128, BS], f32)
        mnh = mpool.tile([128, BS], f32)

        for b in range(BS):
            mxa = mpool.tile([128, TT * OB], f32, tag=f"mxa{b}")
            mna = mpool.tile([128, TT * OB], f32, tag=f"mna{b}")
            for t in range(TT):
                t0 = t * NT
                if (b, t) == (0, 0):
                    xh, xl = first_xhl
                elif (b, t) == (0, 1):
                    xh, xl = second_xhl
                else:
                    xh, xl = load_x(b, t)

                for ob in range(OB):
                    ps = pspool.tile([128, NT], f32, tag="ps")
                    for p in range(NPAIR):
                        wap = wsb[:, (ob * NPAIR + p) * 128:(ob * NPAIR + p + 1) * 128]
                        nc.tensor.matmul(ps[:], wap, xh[:, 2 * p:2 * p + NT],
                                         start=(p == 0), stop=False)
                        nc.tensor.matmul(ps[:], wap, xl[:, 2 * p:2 * p + NT],
                                         start=False, stop=(p == NPAIR - 1))

                    ot = opool.tile([128, NT], f32, tag="ot")
                    nc.scalar.activation(ot[:], ps[:],
                                         mybir.ActivationFunctionType.Identity,
                                         bias=bsb[:, ob:ob + 1], scale=float(scale_w))
                    idx = t * OB + ob
                    nc.vector.tensor_reduce(mxa[:, idx:idx + 1], ot[:],
                                            axis=mybir.AxisListType.X,
                                            op=mybir.AluOpType.max)
                    nc.vector.tensor_reduce(mna[:, idx:idx + 1], ot[:],
                                            axis=mybir.AxisListType.X,
                                            op=mybir.AluOpType.min)
                    nc.sync.dma_start(y_d[b, ob * 128:(ob + 1) * 128, t0:t0 + NT], ot[:])
            nc.vector.tensor_reduce(mxh[:, b:b + 1], mxa[:], axis=mybir.AxisListType.X,
                                    op=mybir.AluOpType.max)
            nc.vector.tensor_reduce(mnh[:, b:b + 1], mna[:], axis=mybir.AxisListType.X,
                                    op=mybir.AluOpType.min)

        mmt = mpool.tile([128, 2], f32)
        nc.vector.tensor_reduce(mmt[:, 0:1], mxh[:], axis=mybir.AxisListType.X,
                                op=mybir.AluOpType.max)
        nc.vector.tensor_reduce(mmt[:, 1:2], mnh[:], axis=mybir.AxisListType.X,
                                op=mybir.AluOpType.min)
        nc.sync.dma_start(mm_d[:], mmt[:])

    nc.compile()
    return nc


def _get_compiled(pad: int, scale_w: float):
    key = (pad, float(scale_w))
    if key not in _COMPILED:
        _COMPILED[key] = _build(pad, scale_w)
    return _COMPILED[key]


def _prepare(inputs: dict):
    x = np.ascontiguousarray(np.asarray(inputs["x"], np.float32))
    weight = np.asarray(inputs["weight"], np.float32)
    bias = np.asarray(inputs["bias"], np.float32)
    nd = int(np.asarray(inputs["n_discard"]))
    assert x.shape == (B, C, T) and weight.shape == (O, C * K) and bias.shape == (O,)

    kw, scale_w = _fake_quant_params(weight, 8)   # qw = kw * scale_w
    qb_codes, scale_b = _fake_quant_params(bias, 16)
    qb = (qb_codes * scale_b).astype(np.float32)  # exact: power-of-2 scale

    # integer weight codes |kw| <= 128 are always exact in fp16
    assert np.max(np.abs(kw)) <= 128
    k3 = kw.reshape(O, C, K)
    wts = np.empty((NQ, 128, 128), np.float16)
    for ob in range(OB):
        for p in range(NPAIR):
            blk = k3[ob * 128:(ob + 1) * 128]          # [128, C, K]
            wts[ob * NPAIR + p, 0:64, :] = blk[:, :, 2 * p].T
            wts[ob * NPAIR + p, 64:128, :] = blk[:, :, 2 * p + 1].T
    # chunked layout for the device: 4 consecutive blocks side by side per row
    wts = np.ascontiguousarray(
        wts.reshape(NQ // 4, 4, 128, 128).transpose(0, 2, 1, 3).reshape(NQ // 4, 128, 512))

    qb2 = np.ascontiguousarray(qb.reshape(OB, 128).T)  # [128, OB]

    pad = K - 1 + nd
    in_maps = [
        {"x": np.ascontiguousarray(x[i * BS:(i + 1) * BS]),
         "wts": wts, "qb": qb2}
        for i in range(N_CORES)
    ]
    return in_maps, pad, float(scale_w)


def _postprocess(results):
    y = np.concatenate([r["y"] for r in results], axis=0)
    mx = max(float(r["mm"][:, 0].max()) for r in results)
    mn = min(float(r["mm"][:, 1].min()) for r in results)
    maxabs = np.float32(max(mx, -mn))
    # output requant: scale = 2^(ceil(log2(max|y|+1e-12)) - 15); without
    # rounding, q*scale == y exactly (power-of-2 scale) except where the
    # clip binds, which requires max|y| within a factor 32768/32767 of a
    # power of two.
    quanta = _ceil_log2_f32(np.float32(maxabs + np.float32(1e-12))) - 15
    scale = np.float32(np.exp2(np.float32(quanta)))
    hi = np.float32(np.float32(32767.0) * scale)
    lo = np.float32(np.float32(-32768.0) * scale)
    if maxabs > hi:
        np.minimum(y, hi, out=y)
    if -maxabs < lo:
        np.maximum(y, lo, out=y)
    return y


def _run(inputs: dict, trace: bool = False, **kwargs):
    from concourse.bass_utils import run_bass_kernel_spmd

    in_maps, pad, scale_w = _prepare(inputs)
    nc = _get_compiled(pad, scale_w)
    bkr = run_bass_kernel_spmd(nc, in_maps, list(range(N_CORES)), trace=trace,
                               **kwargs)
    y = _postprocess(bkr.results)
    return y, bkr


def kernel(**inputs) -> np.ndarray:
    y, _ = _run(inputs, trace=False)
    return y


def run_traced(inputs, **kwargs):
    return _run(inputs, trace=True, **kwargs)


# revision 11
# speedup vs baseline: 1.0013x; 1.0013x over previous
"""Trainium2 Bass kernel for nn_CumulativeFlattenedLinear.

reference:
  y = fq_out( causal_conv1d(x, fq8(w).reshape(O,C,K), extra_delay=n_discard)
              + fq16(b) )
with power-of-2-scale fake quantization.

Sharding: data-parallel over batch B=16 -> 2 batches per core on 8 cores.
Weight (2 MB) replicated.

On-device compute per core:
  - conv as PE matmuls, contract dim 128 = 64 ch x 2 adjacent taps.
    Partitions 0..63 of the rhs tile hold x's window; partitions 64..127
    hold the same window shifted by one sample, so tap pair (2p, 2p+1)
    is one matmul whose rhs is a column-slice of a single SBUF tile.
    16 tap-pairs accumulate into one PSUM bank per [128 o x 512 t] tile.
  - weights are 8-bit power-of-2 fake-quantized: their integer codes are
    always exact in fp16. x is split x = hi + lo (fp16 each, ~22 mantissa
    bits). int8 x fp16 products are exact in the PE multiplier and
    accumulate in fp32 PSUM, so two fp16 passes reproduce an fp32 conv to
    accumulation-rounding accuracy at half the PE cost of native fp32
    (which takes 4 array passes).
  - epilogue on ACT: out = psum * 2^quanta_w + qbias; DVE tracks
    per-partition running max/min of y for the output requant scale.

Host: weight/bias fake-quant (tiny), final max combine across cores, and
the output requant clip - a no-op unless max|y| sits within 2^-15 of a
power of two (then applied exactly with np.minimum).
"""

import numpy as np

B, C, T, O, K = 16, 64, 8192, 256, 32
N_CORES = 8
BS = B // N_CORES          # batches per core
NT = 512                   # time-tile (one PSUM bank of fp32)
TT = T // NT               # time tiles
OB = O // 128              # output-channel blocks
NPAIR = K // 2             # tap pairs per output tile
NQ = OB * NPAIR            # distinct 128x128 weight blocks
W_WIN = NT + K - 2         # x window columns needed per time tile (542)

_COMPILED: dict = {}


def _ceil_log2_f32(m: np.float32) -> float:
    # exact ceil(log2(m)) for finite positive fp32 m, matching
    # jnp.ceil(jnp.log2(m)) for every case where log2 is correctly rounded
    mant, ex = np.frexp(np.float32(m))  # m = mant * 2^ex, mant in [0.5, 1)
    return float(ex - 1) if mant == 0.5 else float(ex)


def _fake_quant_params(w: np.ndarray, bits: int):
    """Return (q_codes_f32, scale_f32) mirroring reference.fake_quantize:
    fq = clip(round(w/scale), lo, hi) * scale, scale = 2^(ceil(log2(max|w|+1e-12)) - (bits-1))."""
    w32 = np.asarray(w, np.float32)
    maxabs = np.float32(np.max(np.abs(w32)))
    quanta = _ceil_log2_f32(np.float32(maxabs + np.float32(1e-12))) - (bits - 1)
    scale = np.float32(np.exp2(np.float32(quanta)))
    q = (w32 / scale).astype(np.float32)
    q = np.round(q)  # RNE, same as jnp.round
    lo = float(-(2 ** (bits - 1)))
    hi = float(2 ** (bits - 1) - 1)
    q = np.clip(q, lo, hi).astype(np.float32)
    return q, scale


def _build(pad: int, scale_w: float):
    from contextlib import ExitStack

    import concourse.bass as bass
    import concourse.tile as tile
    from concourse import bacc, mybir

    f32 = mybir.dt.float32
    f16 = mybir.dt.float16

    nc = bacc.Bacc(
        "TRN2",
        target_bir_lowering=False,
        debug=False,
        enable_asserts=False,
        num_devices=N_CORES,
    )

    x_d = nc.dram_tensor("x", [BS, C, T], f32, kind="ExternalInput").ap()
    w_d = nc.dram_tensor("wts", [NQ // 4, 128, 512], f16, kind="ExternalInput").ap()
    b_d = nc.dram_tensor("qb", [128, OB], f32, kind="ExternalInput").ap()
    y_d = nc.dram_tensor("y", [BS, O, T], f32, kind="ExternalOutput").ap()
    mm_d = nc.dram_tensor("mm", [128, 2], f32, kind="ExternalOutput").ap()

    NTILES = BS * TT * OB  # output tiles -> columns in the max/min accumulators

    with tile.TileContext(nc) as tc, ExitStack() as ctx:
        wpool = ctx.enter_context(tc.tile_pool(name="w", bufs=1))
        bpool = ctx.enter_context(tc.tile_pool(name="b", bufs=1))
        mpool = ctx.enter_context(tc.tile_pool(name="mx", bufs=1))
        xpool = ctx.enter_context(tc.tile_pool(name="x", bufs=3))
        hpool = ctx.enter_context(tc.tile_pool(name="xh", bufs=3))
        lpool = ctx.enter_context(tc.tile_pool(name="xl", bufs=3))
        opool = ctx.enter_context(tc.tile_pool(name="out", bufs=4))
        pspool = ctx.enter_context(tc.tile_pool(name="ps", bufs=4, space="PSUM"))

        def load_x(b, t, dep=None):
            t0 = t * NT
            xt = xpool.tile([128, W_WIN], f32, tag="xt")
            # rows 0..63   <- x[b, :, t0-pad   : t0-pad+W_WIN]
            # rows 64..127 <- x[b, :, t0-pad+1 : t0-pad+1+W_WIN]
            lo0 = t0 - pad
            # x loads go via gpsimd (SWDGE queues), weights/outputs via sync
            if lo0 < 0:
                nc.vector.memset(xt[:], 0.0)
                d1 = nc.gpsimd.dma_start(xt[0:64, -lo0:W_WIN], x_d[b, :, 0:W_WIN + lo0])
                d2 = nc.gpsimd.dma_start(xt[64:128, -lo0 - 1:W_WIN], x_d[b, :, 0:W_WIN + lo0 + 1])
            else:
                d1 = nc.gpsimd.dma_start(xt[0:64, :], x_d[b, :, lo0:lo0 + W_WIN])
                d2 = nc.gpsimd.dma_start(xt[64:128, :], x_d[b, :, lo0 + 1:lo0 + 1 + W_WIN])
            if dep is not None:
                tile.add_dep_helper(d1.ins, dep.ins, reason="defer to first x tile")
                tile.add_dep_helper(d2.ins, dep.ins, reason="defer to first x tile")
            xh = hpool.tile([128, W_WIN], f16, tag="xh")
            cast = nc.vector.tensor_copy(xh[:], xt[:])
            xl = lpool.tile([128, W_WIN], f16, tag="xl")
            nc.vector.tensor_tensor(xl[:], xt[:], xh[:], mybir.AluOpType.subtract)
            return (xh, xl), cast

        # First x tile gets the DMA engines almost to itself: only weight
        # chunks 0/1 and the bias load alongside it; everything else defers
        # until the first hi-half (CAST) is ready.
        first_xhl, cast0 = load_x(0, 0)
        wsb = wpool.tile([128, NQ * 128], f16)
        nc.sync.dma_start(wsb[:, 0:512], w_d[0])
        nc.sync.dma_start(wsb[:, 512:1024], w_d[1])
        bsb = bpool.tile([128, OB], f32)
        nc.sync.dma_start(bsb[:], b_d[:])
        second_xhl, _ = load_x(0, 1, dep=cast0)
        for cq in range(2, NQ // 4):
            wdma = nc.sync.dma_start(wsb[:, cq * 512:(cq + 1) * 512], w_d[cq])
            if cq == 2:
                tile.add_dep_helper(wdma.ins, cast0.ins, reason="defer to first x tile")

        # per-batch running max/min columns, folded per batch then at the end
        mxh = mpool.tile([128, BS], f32)
        mnh = mpool.tile([128, BS], f32)

        for b in range(BS):
            mxa = mpool.tile([128, TT * OB], f32, tag=f"mxa{b}")
            mna = mpool.tile([128, TT * OB], f32, tag=f"mna{b}")
            for t in range(TT):
                t0 = t * NT
                if (b, t) == (0, 0):
                    xh, xl = first_xhl
                elif (b, t) == (0, 1):
                    xh, xl = second_xhl
                else:
                    (xh, xl), _ = load_x(b, t)

                for ob in range(OB):
                    ps = pspool.tile([128, NT], f32, tag="ps")
                    for p in range(NPAIR):
                        wap = wsb[:, (ob * NPAIR + p) * 128:(ob * NPAIR + p + 1) * 128]
                        nc.tensor.matmul(ps[:], wap, xh[:, 2 * p:2 * p + NT],
                                         start=(p == 0), stop=False)
                        nc.tensor.matmul(ps[:], wap, xl[:, 2 * p:2 * p + NT],
                                         start=False, stop=(p == NPAIR - 1))

                    ot = opool.tile([128, NT], f32, tag="ot")
                    nc.scalar.activation(ot[:], ps[:],
                                         mybir.ActivationFunctionType.Identity,
                                         bias=bsb[:, ob:ob + 1], scale=float(scale_w))
                    idx = t * OB + ob
                    nc.vector.tensor_reduce(mxa[:, idx:idx + 1], ot[:],
                                            axis=mybir.AxisListType.X,
                                            op=mybir.AluOpType.max)
                    nc.vector.tensor_reduce(mna[:, idx:idx + 1], ot[:],
                                            axis=mybir.AxisListType.X,
                                            op=mybir.AluOpType.min)
                    nc.sync.dma_start(y_d[b, ob * 128:(ob + 1) * 128, t0:t0 + NT], ot[:])
            nc.vector.tensor_reduce(mxh[:, b:b + 1], mxa[:], axis=mybir.AxisListType.X,
                                    op=mybir.AluOpType.max)
            nc.vector.tensor_reduce(mnh[:, b:b + 1], mna[:], axis=mybir.AxisListType.X,
                                    op=mybir.AluOpType.min)

        mmt = mpool.tile([128, 2], f32)
        nc.vector.tensor_reduce(mmt[:, 0:1], mxh[:], axis=mybir.AxisListType.X,
                                op=mybir.AluOpType.max)
        nc.vector.tensor_reduce(mmt[:, 1:2], mnh[:], axis=mybir.AxisListType.X,
                                op=mybir.AluOpType.min)
        nc.sync.dma_start(mm_d[:], mmt[:])

    nc.compile()
    return nc


def _get_compiled(pad: int, scale_w: float):
    key = (pad, float(scale_w))
    if key not in _COMPILED:
        _COMPILED[key] = _build(pad, scale_w)
    return _COMPILED[key]


def _prepare(inputs: dict):
    x = np.ascontiguousarray(np.asarray(inputs["x"], np.float32))
    weight = np.asarray(inputs["weight"], np.float32)
    bias = np.asarray(inputs["bias"], np.float32)
    nd = int(np.asarray(inputs["n_discard"]))
    assert x.shape == (B, C, T) and weight.shape == (O, C * K) and bias.shape == (O,)

    kw, scale_w = _fake_quant_params(weight, 8)   # qw = kw * scale_w
    qb_codes, scale_b = _fake_quant_params(bias, 16)
    qb = (qb_codes * scale_b).astype(np.float32)  # exact: power-of-2 scale

    # integer weight codes |kw| <= 128 are always exact in fp16
    assert np.max(np.abs(kw)) <= 128
    k3 = kw.reshape(O, C, K)
    wts = np.empty((NQ, 128, 128), np.float16)
    for ob in range(OB):
        for p in range(NPAIR):
            blk = k3[ob * 128:(ob + 1) * 128]          # [128, C, K]
            wts[ob * NPAIR + p, 0:64, :] = blk[:, :, 2 * p].T
            wts[ob * NPAIR + p, 64:128, :] = blk[:, :, 2 * p + 1].T
    # chunked layout for the device: 4 consecutive blocks side by side per row
    wts = np.ascontiguousarray(
        wts.reshape(NQ // 4, 4, 128, 128).transpose(0, 2, 1, 3).reshape(NQ // 4, 128, 512))

    qb2 = np.ascontiguousarray(qb.reshape(OB, 128).T)  # [128, OB]

    pad = K - 1 + nd
    in_maps = [
        {"x": np.ascontiguousarray(x[i * BS:(i + 1) * BS]),
         "wts": wts, "qb": qb2}
        for i in range(N_CORES)
    ]
    return in_maps, pad, float(scale_w)


def _postprocess(results):
    y = np.concatenate([r["y"] for r in results], axis=0)
    mx = max(float(r["mm"][:, 0].max()) for r in results)
    mn = min(float(r["mm"][:, 1].min()) for r in results)
    maxabs = np.float32(max(mx, -mn))
    # output requant: scale = 2^(ceil(log2(max|y|+1e-12)) - 15); without
    # rounding, q*scale == y exactly (power-of-2 scale) except where the
    # clip binds, which requires max|y| within a factor 32768/32767 of a
    # power of two.
    quanta = _ceil_log2_f32(np.float32(maxabs + np.float32(1e-12))) - 15
    scale = np.float32(np.exp2(np.float32(quanta)))
    hi = np.float32(np.float32(32767.0) * scale)
    lo = np.float32(np.float32(-32768.0) * scale)
    if maxabs > hi:
        np.minimum(y, hi, out=y)
    if -maxabs < lo:
        np.maximum(y, lo, out=y)
    return y


def _run(inputs: dict, trace: bool = False, **kwargs):
    from concourse.bass_utils import run_bass_kernel_spmd

    in_maps, pad, scale_w = _prepare(inputs)
    nc = _get_compiled(pad, scale_w)
    bkr = run_bass_kernel_spmd(nc, in_maps, list(range(N_CORES)), trace=trace,
                               **kwargs)
    y = _postprocess(bkr.results)
    return y, bkr


def kernel(**inputs) -> np.ndarray:
    y, _ = _run(inputs, trace=False)
    return y


def run_traced(inputs, **kwargs):
    return _run(inputs, trace=True, **kwargs)
